# revision 5
# baseline (speedup 1.0000x reference)
"""Chamfer loss kernel for Trainium2 (8 NeuronCores).

Strategy
--------
B=4 batches, K=8192 points, 3D coords. 8 cores = (batch b, half h) pairs:
core c -> b = c//2, h = c%2. Each core handles two "orientations" for its
batch:
  A: queries = pred[b, half h] (4096), refs = target[b] (8192)
  B: queries = target[b, half h] (4096), refs = pred[b] (8192)
Within an orientation, for each query i we need min_j ||q_i - r_j||.
Using d2(i,j) = q2_i + r2_j - 2 q_i.r_j = q2_i - s(i,j) with
s(i,j) = 2 q_i . r_j - r2_j, we compute s on the tensor engine as a matmul
(contract dim 11: fp16 hi/lo split of 2q and r, plus hi/lo of r2 with -1
weights -- near-fp32 exact), reduce max_j s on the vector engine with the
fused tensor_tensor_reduce op (2 candidates/cycle: one operand streamed
directly from PSUM, the other staged PSUM->SBUF by the scalar engine), then
d_i = sqrt(relu(q2_i - max_j s)) and masked sums, all batched.

Per core the device returns [128, 3] partial sums (p2t, t2p, mask); the
host sums partitions/cores and forms the final scalar.
"""

import numpy as np

import concourse.bacc as bacc
import concourse.tile as tile
from concourse import mybir
from concourse.bass_utils import run_bass_kernel_spmd

B, K = 4, 8192
HALF = K // 2        # 4096 queries per core per orientation
NT = HALF // 128     # 32 query tiles
NQ = 4               # chunks of 2048 refs (1024 direct + 1024 staged)
F32 = mybir.dt.float32
F16 = mybir.dt.float16
NCORES = 8

_NEG = -3.0e38


def _f16_split(a):
    hi = a.astype(np.float16)
    lo = (a.astype(np.float32) - hi.astype(np.float32)).astype(np.float16)
    return hi, lo


def _build_lhs(q):
    """lhsT [11, n] fp16 for queries q (n,3): rows pair with _build_rhs."""
    a = 2.0 * q.astype(np.float32)
    ahi, alo = _f16_split(a)
    n = q.shape[0]
    out = np.empty((11, n), np.float16)
    out[0:3] = ahi.T
    out[3:6] = ahi.T
    out[6:9] = alo.T
    out[9] = -1.0
    out[10] = -1.0
    return out


def _build_rhs(r):
    """rhs [11, m] fp16 for refs r (m,3)."""
    rf = r.astype(np.float32)
    rhi, rlo = _f16_split(rf)
    r2 = (rf.astype(np.float64) ** 2).sum(-1).astype(np.float32)
    r2hi, r2lo = _f16_split(r2)
    m = r.shape[0]
    out = np.empty((11, m), np.float16)
    out[0:3] = rhi.T
    out[3:6] = rlo.T
    out[6:9] = rhi.T
    out[9] = r2hi
    out[10] = r2lo
    return out


def _cols(v):
    """(HALF,) -> [128, NT] with [p, t] = v[t*128 + p]."""
    return np.ascontiguousarray(v.reshape(NT, 128).T)


def build_nc(reps=1, K_=K, num_devices=NCORES):
    HALF_ = K_ // 2
    NT_ = HALF_ // 128
    NQ_ = max(1, K_ // 2048)
    nc = bacc.Bacc("TRN2", target_bir_lowering=False, debug=False,
                   num_devices=num_devices)
    lhsA_d = nc.dram_tensor("lhsA", [11, HALF_], F16, kind="ExternalInput").ap()
    rhsA_d = nc.dram_tensor("rhsA", [11, K_], F16, kind="ExternalInput").ap()
    lhsB_d = nc.dram_tensor("lhsB", [11, HALF_], F16, kind="ExternalInput").ap()
    rhsB_d = nc.dram_tensor("rhsB", [11, K_], F16, kind="ExternalInput").ap()
    q2A_d = nc.dram_tensor("q2A", [128, NT_], F32, kind="ExternalInput").ap()
    q2B_d = nc.dram_tensor("q2B", [128, NT_], F32, kind="ExternalInput").ap()
    mask_d = nc.dram_tensor("maskc", [128, NT_], F32, kind="ExternalInput").ap()
    sums_d = nc.dram_tensor("sums", [128, 3], F32, kind="ExternalOutput").ap()

    mx = mybir.AluOpType.max
    with tile.TileContext(nc) as tc:
        with (
            tc.tile_pool(name="const", bufs=1) as cpool,
            tc.tile_pool(name="psD", bufs=2, space="PSUM") as psD,
            tc.tile_pool(name="psS", bufs=2, space="PSUM") as psS,
            tc.tile_pool(name="stg", bufs=3) as stg,
            tc.tile_pool(name="scr", bufs=2) as scr,
            tc.tile_pool(name="fin", bufs=1) as fin,
        ):
            lhsA = cpool.tile([11, HALF_], F16, tag="lhsA")
            nc.sync.dma_start(lhsA[:], lhsA_d)
            rhsA = cpool.tile([11, K_], F16, tag="rhsA")
            nc.sync.dma_start(rhsA[:], rhsA_d)
            lhsB = cpool.tile([11, HALF_], F16, tag="lhsB")
            nc.sync.dma_start(lhsB[:], lhsB_d)
            rhsB = cpool.tile([11, K_], F16, tag="rhsB")
            nc.sync.dma_start(rhsB[:], rhsB_d)
            q2A = cpool.tile([128, NT_], F32, tag="q2A")
            nc.sync.dma_start(q2A[:], q2A_d)
            q2B = cpool.tile([128, NT_], F32, tag="q2B")
            nc.sync.dma_start(q2B[:], q2B_d)
            maskc = cpool.tile([128, NT_], F32, tag="maskc")
            nc.sync.dma_start(maskc[:], mask_d)
            resA = cpool.tile([128, NT_], F32, tag="resA")
            resB = cpool.tile([128, NT_], F32, tag="resB")
            sums = cpool.tile([128, 3], F32, tag="sums")

            for _ in range(reps):
                for lhs, rhs, res in ((lhsA, rhsA, resA), (lhsB, rhsB, resB)):
                    for t in range(NT_):
                        lw = lhs[:, t * 128:(t + 1) * 128]
                        sc_prev = None
                        for q in range(NQ_):
                            base = q * 2048
                            dt_ = psD.tile([128, 1024], F32, tag="d")
                            st_ = psS.tile([128, 1024], F32, tag="s")
                            nc.tensor.matmul(dt_[:, 0:512], lw,
                                             rhs[:, base:base + 512])
                            nc.tensor.matmul(dt_[:, 512:1024], lw,
                                             rhs[:, base + 512:base + 1024])
                            nc.tensor.matmul(st_[:, 0:512], lw,
                                             rhs[:, base + 1024:base + 1536])
                            nc.tensor.matmul(st_[:, 512:1024], lw,
                                             rhs[:, base + 1536:base + 2048])
                            sg = stg.tile([128, 1024], F32, tag="sg")
                            nc.scalar.copy(sg[:], st_[:])
                            sc = scr.tile([128, 1024], F32, tag="sc")
                            nc.vector.tensor_tensor_scan(
                                out=sc[:], data0=dt_[:], data1=sg[:],
                                initial=(_NEG if q == 0
                                         else sc_prev[:, 1023:1024]),
                                op0=mx, op1=mx,
                            )
                            sc_prev = sc
                        nc.scalar.copy(res[:, t:t + 1],
                                       sc_prev[:, 1023:1024])

                for res, q2, col in ((resA, q2A, 0), (resB, q2B, 1)):
                    d2 = fin.tile([128, NT_], F32, tag="d2")
                    nc.vector.tensor_sub(d2[:], q2[:], res[:])
                    d2c = fin.tile([128, NT_], F32, tag="d2c")
                    nc.vector.tensor_scalar_max(d2c[:], d2[:], 0.0)
                    dd = fin.tile([128, NT_], F32, tag="dd")
                    nc.scalar.activation(dd[:], d2c[:],
                                         mybir.ActivationFunctionType.Sqrt)
                    dm = fin.tile([128, NT_], F32, tag="dm")
                    nc.vector.tensor_mul(dm[:], dd[:], maskc[:])
                    nc.vector.tensor_reduce(sums[:, col:col + 1], dm[:],
                                            axis=mybir.AxisListType.X,
                                            op=mybir.AluOpType.add)
                nc.vector.tensor_reduce(sums[:, 2:3], maskc[:],
                                        axis=mybir.AxisListType.X,
                                        op=mybir.AluOpType.add)
            nc.sync.dma_start(sums_d, sums[:])
    nc.compile()
    return nc


def make_in_maps(pred, target, mask):
    pred = np.asarray(pred, np.float32)
    target = np.asarray(target, np.float32)
    mask = np.asarray(mask, np.float32)
    in_maps = []
    for c in range(NCORES):
        b, h = c // 2, c % 2
        sl = slice(h * HALF, (h + 1) * HALF)
        pq = pred[b, sl]
        tq = target[b, sl]
        in_maps.append({
            "lhsA": _build_lhs(pq),
            "rhsA": _build_rhs(target[b]),
            "lhsB": _build_lhs(tq),
            "rhsB": _build_rhs(pred[b]),
            "q2A": _cols((pq.astype(np.float64) ** 2).sum(-1)
                         .astype(np.float32)),
            "q2B": _cols((tq.astype(np.float64) ** 2).sum(-1)
                         .astype(np.float32)),
            "maskc": _cols(mask[b, sl]),
        })
    return in_maps


def combine(results):
    s = np.stack([np.asarray(r["sums"], np.float64) for r in results])
    tot = s.sum(axis=(0, 1))  # [p2t_sum, t2p_sum, mask_sum]
    denom = tot[2] + 1e-8
    return np.float32((tot[0] / denom + tot[1] / denom) / 2.0)


_NC_CACHE = {}


def _get_nc(reps=1):
    if reps not in _NC_CACHE:
        _NC_CACHE[reps] = build_nc(reps)
    return _NC_CACHE[reps]


def kernel(pred, target, mask):
    nc = _get_nc(1)
    in_maps = make_in_maps(pred, target, mask)
    res = run_bass_kernel_spmd(nc, in_maps, list(range(NCORES)))
    return combine(res.results)


# revision 15
# speedup vs baseline: 194.6316x; 194.6316x over previous
"""Chamfer loss kernel for Trainium2 (8 NeuronCores).

Strategy
--------
B=4 batches, K=8192 points, 3D coords. 8 cores = (batch b, half h) pairs:
core c -> b = c//2, h = c%2. Each core handles two "orientations" for its
batch:
  A: queries = pred[b, half h] (4096), refs = target[b] (8192)
  B: queries = target[b, half h] (4096), refs = pred[b] (8192)
Within an orientation, for each query i we need min_j ||q_i - r_j||.
Using d2(i,j) = q2_i + r2_j - 2 q_i.r_j = q2_i - s(i,j) with
s(i,j) = 2 q_i . r_j - r2_j, we compute s on the tensor engine as a matmul
(contract dim 11: fp16 hi/lo split of 2q and r, plus hi/lo of r2 with -1
weights -- near-fp32 exact; fp16 streams 1 col/cycle vs fp32's 4 cycles).
The direct-chunk and staged-chunk matmuls are placed on different PE
row-groups (tile_position (0,0) / (32,0), operands replicated at SBUF
partitions 0-10 and 32-42) so they execute concurrently on the array.
max_j s is reduced on the vector engine with tensor_tensor_scan
(op0=op1=max: running max over TWO streams at once -- one operand read
directly from PSUM, the other staged PSUM->SBUF by the scalar engine;
chained across chunks via initial=prev[:, -1:]). Finally
d_i = sqrt(relu(q2_i - max_j s)) and masked sums, batched per orientation.

Per core the device returns [128, 3] partial sums (p2t, t2p, mask); the
host sums partitions/cores and forms the final scalar.

Notes from HW measurement (no NTFF profiling in this container; timing by
on-device For_i repetition slope): ~520-610 us/iteration, DVE-scan bound
(scan ~= 1.1 cyc/candidate incl. staging; ACT staging ~80% busy; packed
PE ~110 us). tensor_tensor_reduce (the nominally ideal fused op) crashes
the device here (NRT_EXEC_UNIT_UNRECOVERABLE); gpsimd.tensor_tensor does
not compile in this walrus -- hence the scan-based design.
"""

import numpy as np

import concourse.bacc as bacc
import concourse.tile as tile
from concourse import mybir
from concourse.bass_utils import run_bass_kernel_spmd

B, K = 4, 8192
HALF = K // 2        # 4096 queries per core per orientation
NT = HALF // 128     # 32 query tiles
NQ = 4               # chunks of 2048 refs (1024 direct + 1024 staged)
F32 = mybir.dt.float32
F16 = mybir.dt.float16
NCORES = 8

_NEG = -3.0e38


def _f16_split(a):
    hi = a.astype(np.float16)
    lo = (a.astype(np.float32) - hi.astype(np.float32)).astype(np.float16)
    return hi, lo


def _build_lhs(q):
    """lhsT [11, n] fp16 for queries q (n,3): rows pair with _build_rhs."""
    a = 2.0 * q.astype(np.float32)
    ahi, alo = _f16_split(a)
    n = q.shape[0]
    out = np.empty((11, n), np.float16)
    out[0:3] = ahi.T
    out[3:6] = ahi.T
    out[6:9] = alo.T
    out[9] = -1.0
    out[10] = -1.0
    return out


def _build_rhs(r):
    """rhs [11, m] fp16 for refs r (m,3)."""
    rf = r.astype(np.float32)
    rhi, rlo = _f16_split(rf)
    r2 = (rf.astype(np.float64) ** 2).sum(-1).astype(np.float32)
    r2hi, r2lo = _f16_split(r2)
    m = r.shape[0]
    out = np.empty((11, m), np.float16)
    out[0:3] = rhi.T
    out[3:6] = rlo.T
    out[6:9] = rhi.T
    out[9] = r2hi
    out[10] = r2lo
    return out


def _cols(v):
    """(HALF,) -> [128, NT] with [p, t] = v[t*128 + p]."""
    return np.ascontiguousarray(v.reshape(NT, 128).T)


def build_nc(reps=1, K_=K, num_devices=NCORES, loop_reps=0):
    HALF_ = K_ // 2
    NT_ = HALF_ // 128
    NQ_ = max(1, K_ // 2048)
    nc = bacc.Bacc("TRN2", target_bir_lowering=False, debug=False,
                   num_devices=num_devices)
    lhsA_d = nc.dram_tensor("lhsA", [11, HALF_], F16, kind="ExternalInput").ap()
    rhsA_d = nc.dram_tensor("rhsA", [11, K_], F16, kind="ExternalInput").ap()
    lhsB_d = nc.dram_tensor("lhsB", [11, HALF_], F16, kind="ExternalInput").ap()
    rhsB_d = nc.dram_tensor("rhsB", [11, K_], F16, kind="ExternalInput").ap()
    # The direct-chunk matmuls run on PE row-group 0 (SBUF partitions 0-10)
    # and the staged-chunk matmuls on row-group 1 (partitions 32-42), so the
    # two streams execute concurrently on the systolic array (2x PE rate).
    # lhs/rhs are replicated into both partition ranges of a [43, n] tile.
    q2A_d = nc.dram_tensor("q2A", [128, NT_], F32, kind="ExternalInput").ap()
    q2B_d = nc.dram_tensor("q2B", [128, NT_], F32, kind="ExternalInput").ap()
    mask_d = nc.dram_tensor("maskc", [128, NT_], F32, kind="ExternalInput").ap()
    sums_d = nc.dram_tensor("sums", [128, 3], F32, kind="ExternalOutput").ap()

    mx = mybir.AluOpType.max
    with tile.TileContext(nc) as tc:
        with (
            tc.tile_pool(name="const", bufs=1) as cpool,
            tc.tile_pool(name="psD", bufs=2, space="PSUM") as psD,
            tc.tile_pool(name="psS", bufs=2, space="PSUM") as psS,
            tc.tile_pool(name="stg", bufs=4) as stg,
            tc.tile_pool(name="scr", bufs=4) as scr,
            tc.tile_pool(name="fin", bufs=1) as fin,
        ):
            lhsA = cpool.tile([43, HALF_], F16, tag="lhsA")
            nc.sync.dma_start(lhsA[0:11, :], lhsA_d)
            nc.sync.dma_start(lhsA[32:43, :], lhsA_d)
            rhsA = cpool.tile([43, K_], F16, tag="rhsA")
            nc.sync.dma_start(rhsA[0:11, :], rhsA_d)
            nc.sync.dma_start(rhsA[32:43, :], rhsA_d)
            lhsB = cpool.tile([43, HALF_], F16, tag="lhsB")
            nc.sync.dma_start(lhsB[0:11, :], lhsB_d)
            nc.sync.dma_start(lhsB[32:43, :], lhsB_d)
            rhsB = cpool.tile([43, K_], F16, tag="rhsB")
            nc.sync.dma_start(rhsB[0:11, :], rhsB_d)
            nc.sync.dma_start(rhsB[32:43, :], rhsB_d)
            q2A = cpool.tile([128, NT_], F32, tag="q2A")
            nc.sync.dma_start(q2A[:], q2A_d)
            q2B = cpool.tile([128, NT_], F32, tag="q2B")
            nc.sync.dma_start(q2B[:], q2B_d)
            maskc = cpool.tile([128, NT_], F32, tag="maskc")
            nc.sync.dma_start(maskc[:], mask_d)
            resA = cpool.tile([128, NT_], F32, tag="resA")
            resB = cpool.tile([128, NT_], F32, tag="resB")
            sums = cpool.tile([128, 3], F32, tag="sums")

            def body():
                for lhs, rhs, res in ((lhsA, rhsA, resA), (lhsB, rhsB, resB)):
                    for t in range(NT_):
                        ts0, ts1 = t * 128, (t + 1) * 128
                        lw0 = lhs[0:11, ts0:ts1]
                        lw1 = lhs[32:43, ts0:ts1]
                        sc_prevs = [None, None]
                        for q in range(NQ_):
                            base = q * 2048
                            dt_ = psD.tile([128, 1024], F32, tag="d")
                            st_ = psS.tile([128, 1024], F32, tag="s")
                            nc.tensor.matmul(dt_[:, 0:512], lw0,
                                             rhs[0:11, base:base + 512],
                                             tile_position=(0, 0))
                            nc.tensor.matmul(st_[:, 0:512], lw1,
                                             rhs[32:43, base + 1024:base + 1536],
                                             tile_position=(32, 0))
                            nc.tensor.matmul(dt_[:, 512:1024], lw0,
                                             rhs[0:11, base + 512:base + 1024],
                                             tile_position=(0, 0))
                            nc.tensor.matmul(st_[:, 512:1024], lw1,
                                             rhs[32:43, base + 1536:base + 2048],
                                             tile_position=(32, 0))
                            sg = stg.tile([128, 1024], F32, tag="sg")
                            nc.scalar.copy(sg[:], st_[:])
                            sc = scr.tile([128, 1024], F32, tag="sc")
                            prev = sc_prevs[q % 2]
                            nc.vector.tensor_tensor_scan(
                                out=sc[:], data0=dt_[:], data1=sg[:],
                                initial=(_NEG if prev is None
                                         else prev[:, 1023:1024]),
                                op0=mx, op1=mx,
                            )
                            sc_prevs[q % 2] = sc
                        if sc_prevs[1] is None:
                            nc.scalar.copy(res[:, t:t + 1],
                                           sc_prevs[0][:, 1023:1024])
                        else:
                            nc.vector.tensor_max(res[:, t:t + 1],
                                                 sc_prevs[0][:, 1023:1024],
                                                 sc_prevs[1][:, 1023:1024])

                for res, q2, col in ((resA, q2A, 0), (resB, q2B, 1)):
                    d2 = fin.tile([128, NT_], F32, tag="d2")
                    nc.vector.tensor_sub(d2[:], q2[:], res[:])
                    d2c = fin.tile([128, NT_], F32, tag="d2c")
                    nc.vector.tensor_scalar_max(d2c[:], d2[:], 0.0)
                    dd = fin.tile([128, NT_], F32, tag="dd")
                    nc.scalar.activation(dd[:], d2c[:],
                                         mybir.ActivationFunctionType.Sqrt)
                    dm = fin.tile([128, NT_], F32, tag="dm")
                    nc.vector.tensor_mul(dm[:], dd[:], maskc[:])
                    nc.vector.tensor_reduce(sums[:, col:col + 1], dm[:],
                                            axis=mybir.AxisListType.X,
                                            op=mybir.AluOpType.add)
                nc.vector.tensor_reduce(sums[:, 2:3], maskc[:],
                                        axis=mybir.AxisListType.X,
                                        op=mybir.AluOpType.add)

            if loop_reps:
                with tc.For_i(0, loop_reps, 1):
                    body()
            else:
                for _ in range(reps):
                    body()
            nc.sync.dma_start(sums_d, sums[:])
    nc.compile()
    return nc


def make_in_maps(pred, target, mask):
    pred = np.asarray(pred, np.float32)
    target = np.asarray(target, np.float32)
    mask = np.asarray(mask, np.float32)
    in_maps = []
    for c in range(NCORES):
        b, h = c // 2, c % 2
        sl = slice(h * HALF, (h + 1) * HALF)
        pq = pred[b, sl]
        tq = target[b, sl]
        in_maps.append({
            "lhsA": _build_lhs(pq),
            "rhsA": _build_rhs(target[b]),
            "lhsB": _build_lhs(tq),
            "rhsB": _build_rhs(pred[b]),
            "q2A": _cols((pq.astype(np.float64) ** 2).sum(-1)
                         .astype(np.float32)),
            "q2B": _cols((tq.astype(np.float64) ** 2).sum(-1)
                         .astype(np.float32)),
            "maskc": _cols(mask[b, sl]),
        })
    return in_maps


def combine(results):
    s = np.stack([np.asarray(r["sums"], np.float64) for r in results])
    tot = s.sum(axis=(0, 1))  # [p2t_sum, t2p_sum, mask_sum]
    denom = tot[2] + 1e-8
    return np.float32((tot[0] / denom + tot[1] / denom) / 2.0)


_NC_CACHE = {}


def _get_nc(reps=1):
    if reps not in _NC_CACHE:
        _NC_CACHE[reps] = build_nc(reps)
    return _NC_CACHE[reps]


def kernel(pred, target, mask):
    nc = _get_nc(1)
    in_maps = make_in_maps(pred, target, mask)
    res = run_bass_kernel_spmd(nc, in_maps, list(range(NCORES)))
    return combine(res.results)


# revision 16
# speedup vs baseline: 238.2911x; 1.2243x over previous
"""Chamfer loss kernel for Trainium2 (8 NeuronCores).

Strategy
--------
B=4 batches, K=8192 points, 3D coords. 8 cores = (batch b, half h) pairs:
core c -> b = c//2, h = c%2. Each core handles two "orientations" for its
batch:
  A: queries = pred[b, half h] (4096), refs = target[b] (8192)
  B: queries = target[b, half h] (4096), refs = pred[b] (8192)
Within an orientation, for each query i we need min_j ||q_i - r_j||.
Using d2(i,j) = q2_i + r2_j - 2 q_i.r_j = q2_i - s(i,j) with
s(i,j) = 2 q_i . r_j - r2_j, we compute s on the tensor engine as a matmul
(contract dim 11: fp16 hi/lo split of 2q and r, plus hi/lo of r2 with -1
weights -- near-fp32 exact; fp16 streams 1 col/cycle vs fp32's 4 cycles).
The direct-chunk and staged-chunk matmuls are placed on different PE
row-groups (tile_position (0,0) / (32,0), operands replicated at SBUF
partitions 0-10 and 32-42) so they execute concurrently on the array.
max_j s is reduced on the vector engine with tensor_tensor_scan
(op0=op1=max: running max over TWO streams at once -- one operand read
directly from PSUM, the other staged PSUM->SBUF by the scalar engine;
chained across chunks via initial=prev[:, -1:]). Finally
d_i = sqrt(relu(q2_i - max_j s)) and masked sums, batched per orientation.

Per core the device returns [128, 3] partial sums (p2t, t2p, mask); the
host sums partitions/cores and forms the final scalar.

Notes from HW measurement (no NTFF profiling in this container; timing by
on-device For_i repetition slope): ~520-610 us/iteration, DVE-scan bound
(scan ~= 1.1 cyc/candidate incl. staging; ACT staging ~80% busy; packed
PE ~110 us). tensor_tensor_reduce (the nominally ideal fused op) crashes
the device here (NRT_EXEC_UNIT_UNRECOVERABLE); gpsimd.tensor_tensor does
not compile in this walrus -- hence the scan-based design.
"""

import numpy as np

import concourse.bacc as bacc
import concourse.tile as tile
from concourse import mybir
from concourse.bass_utils import run_bass_kernel_spmd

B, K = 4, 8192
HALF = K // 2        # 4096 queries per core per orientation
NT = HALF // 128     # 32 query tiles
NQ = 4               # chunks of 2048 refs (1024 direct + 1024 staged)
F32 = mybir.dt.float32
F16 = mybir.dt.float16
NCORES = 8

_NEG = -3.0e38


def _f16_split(a):
    hi = a.astype(np.float16)
    lo = (a.astype(np.float32) - hi.astype(np.float32)).astype(np.float16)
    return hi, lo


def _build_lhs(q):
    """lhsT [11, n] fp16 for queries q (n,3): rows pair with _build_rhs."""
    a = 2.0 * q.astype(np.float32)
    ahi, alo = _f16_split(a)
    n = q.shape[0]
    out = np.empty((11, n), np.float16)
    out[0:3] = ahi.T
    out[3:6] = ahi.T
    out[6:9] = alo.T
    out[9] = -1.0
    out[10] = -1.0
    return out


def _build_rhs(r):
    """rhs [11, m] fp16 for refs r (m,3)."""
    rf = r.astype(np.float32)
    rhi, rlo = _f16_split(rf)
    r2 = (rf.astype(np.float64) ** 2).sum(-1).astype(np.float32)
    r2hi, r2lo = _f16_split(r2)
    m = r.shape[0]
    out = np.empty((11, m), np.float16)
    out[0:3] = rhi.T
    out[3:6] = rlo.T
    out[6:9] = rhi.T
    out[9] = r2hi
    out[10] = r2lo
    return out


def _cols(v):
    """(HALF,) -> [128, NT] with [p, t] = v[t*128 + p]."""
    return np.ascontiguousarray(v.reshape(NT, 128).T)


def build_nc(reps=1, K_=K, num_devices=NCORES, loop_reps=0):
    HALF_ = K_ // 2
    NT_ = HALF_ // 128
    NQ_ = max(1, K_ // 2048)
    nc = bacc.Bacc("TRN2", target_bir_lowering=False, debug=False,
                   num_devices=num_devices)
    lhsA_d = nc.dram_tensor("lhsA", [11, HALF_], F16, kind="ExternalInput").ap()
    rhsA_d = nc.dram_tensor("rhsA", [11, K_], F16, kind="ExternalInput").ap()
    lhsB_d = nc.dram_tensor("lhsB", [11, HALF_], F16, kind="ExternalInput").ap()
    rhsB_d = nc.dram_tensor("rhsB", [11, K_], F16, kind="ExternalInput").ap()
    # The direct-chunk matmuls run on PE row-group 0 (SBUF partitions 0-10)
    # and the staged-chunk matmuls on row-group 1 (partitions 32-42), so the
    # two streams execute concurrently on the systolic array (2x PE rate).
    # lhs/rhs are replicated into both partition ranges of a [43, n] tile.
    q2A_d = nc.dram_tensor("q2A", [128, NT_], F32, kind="ExternalInput").ap()
    q2B_d = nc.dram_tensor("q2B", [128, NT_], F32, kind="ExternalInput").ap()
    mask_d = nc.dram_tensor("maskc", [128, NT_], F32, kind="ExternalInput").ap()
    sums_d = nc.dram_tensor("sums", [128, 3], F32, kind="ExternalOutput").ap()

    mx = mybir.AluOpType.max
    with tile.TileContext(nc) as tc:
        with (
            tc.tile_pool(name="const", bufs=1) as cpool,
            tc.tile_pool(name="psD", bufs=2, space="PSUM") as psD,
            tc.tile_pool(name="psS", bufs=2, space="PSUM") as psS,
            tc.tile_pool(name="stg", bufs=4) as stg,
            tc.tile_pool(name="scr", bufs=2) as scr,
            tc.tile_pool(name="fin", bufs=1) as fin,
        ):
            lhsA = cpool.tile([43, HALF_], F16, tag="lhsA")
            nc.sync.dma_start(lhsA[0:11, :], lhsA_d)
            nc.sync.dma_start(lhsA[32:43, :], lhsA_d)
            rhsA = cpool.tile([43, K_], F16, tag="rhsA")
            nc.sync.dma_start(rhsA[0:11, :], rhsA_d)
            nc.sync.dma_start(rhsA[32:43, :], rhsA_d)
            lhsB = cpool.tile([43, HALF_], F16, tag="lhsB")
            nc.sync.dma_start(lhsB[0:11, :], lhsB_d)
            nc.sync.dma_start(lhsB[32:43, :], lhsB_d)
            rhsB = cpool.tile([43, K_], F16, tag="rhsB")
            nc.sync.dma_start(rhsB[0:11, :], rhsB_d)
            nc.sync.dma_start(rhsB[32:43, :], rhsB_d)
            q2A = cpool.tile([128, NT_], F32, tag="q2A")
            nc.sync.dma_start(q2A[:], q2A_d)
            q2B = cpool.tile([128, NT_], F32, tag="q2B")
            nc.sync.dma_start(q2B[:], q2B_d)
            maskc = cpool.tile([128, NT_], F32, tag="maskc")
            nc.sync.dma_start(maskc[:], mask_d)
            resA = cpool.tile([128, NT_], F32, tag="resA")
            resB = cpool.tile([128, NT_], F32, tag="resB")
            sums = cpool.tile([128, 3], F32, tag="sums")

            def body():
                for lhs, rhs, res in ((lhsA, rhsA, resA), (lhsB, rhsB, resB)):
                    for t in range(NT_):
                        ts0, ts1 = t * 128, (t + 1) * 128
                        lw0 = lhs[0:11, ts0:ts1]
                        lw1 = lhs[32:43, ts0:ts1]
                        cw = scr.tile([128, 1024 * NQ_], F32, tag="cw")
                        for q in range(NQ_):
                            base = q * 2048
                            dt_ = psD.tile([128, 1024], F32, tag="d")
                            st_ = psS.tile([128, 1024], F32, tag="s")
                            nc.tensor.matmul(dt_[:, 0:512], lw0,
                                             rhs[0:11, base:base + 512],
                                             tile_position=(0, 0))
                            nc.tensor.matmul(st_[:, 0:512], lw1,
                                             rhs[32:43, base + 1024:base + 1536],
                                             tile_position=(32, 0))
                            nc.tensor.matmul(dt_[:, 512:1024], lw0,
                                             rhs[0:11, base + 512:base + 1024],
                                             tile_position=(0, 0))
                            nc.tensor.matmul(st_[:, 512:1024], lw1,
                                             rhs[32:43, base + 1536:base + 2048],
                                             tile_position=(32, 0))
                            sg = stg.tile([128, 1024], F32, tag="sg")
                            nc.scalar.copy(sg[:], st_[:])
                            nc.vector.tensor_tensor_scan(
                                out=cw[:, q * 1024:(q + 1) * 1024],
                                data0=dt_[:], data1=sg[:],
                                initial=_NEG, op0=mx, op1=mx,
                            )
                        if NQ_ == 1:
                            nc.scalar.copy(res[:, t:t + 1], cw[:, 1023:1024])
                        else:
                            lasts = cw[:, 1023:1024 * NQ_:1024]
                            nc.vector.tensor_reduce(
                                res[:, t:t + 1], lasts,
                                axis=mybir.AxisListType.X, op=mx)

                for res, q2, col in ((resA, q2A, 0), (resB, q2B, 1)):
                    d2 = fin.tile([128, NT_], F32, tag="d2")
                    nc.vector.tensor_sub(d2[:], q2[:], res[:])
                    d2c = fin.tile([128, NT_], F32, tag="d2c")
                    nc.vector.tensor_scalar_max(d2c[:], d2[:], 0.0)
                    dd = fin.tile([128, NT_], F32, tag="dd")
                    nc.scalar.activation(dd[:], d2c[:],
                                         mybir.ActivationFunctionType.Sqrt)
                    dm = fin.tile([128, NT_], F32, tag="dm")
                    nc.vector.tensor_mul(dm[:], dd[:], maskc[:])
                    nc.vector.tensor_reduce(sums[:, col:col + 1], dm[:],
                                            axis=mybir.AxisListType.X,
                                            op=mybir.AluOpType.add)
                nc.vector.tensor_reduce(sums[:, 2:3], maskc[:],
                                        axis=mybir.AxisListType.X,
                                        op=mybir.AluOpType.add)

            if loop_reps:
                with tc.For_i(0, loop_reps, 1):
                    body()
            else:
                for _ in range(reps):
                    body()
            nc.sync.dma_start(sums_d, sums[:])
    nc.compile()
    return nc


def make_in_maps(pred, target, mask):
    pred = np.asarray(pred, np.float32)
    target = np.asarray(target, np.float32)
    mask = np.asarray(mask, np.float32)
    in_maps = []
    for c in range(NCORES):
        b, h = c // 2, c % 2
        sl = slice(h * HALF, (h + 1) * HALF)
        pq = pred[b, sl]
        tq = target[b, sl]
        in_maps.append({
            "lhsA": _build_lhs(pq),
            "rhsA": _build_rhs(target[b]),
            "lhsB": _build_lhs(tq),
            "rhsB": _build_rhs(pred[b]),
            "q2A": _cols((pq.astype(np.float64) ** 2).sum(-1)
                         .astype(np.float32)),
            "q2B": _cols((tq.astype(np.float64) ** 2).sum(-1)
                         .astype(np.float32)),
            "maskc": _cols(mask[b, sl]),
        })
    return in_maps


def combine(results):
    s = np.stack([np.asarray(r["sums"], np.float64) for r in results])
    tot = s.sum(axis=(0, 1))  # [p2t_sum, t2p_sum, mask_sum]
    denom = tot[2] + 1e-8
    return np.float32((tot[0] / denom + tot[1] / denom) / 2.0)


_NC_CACHE = {}


def _get_nc(reps=1):
    if reps not in _NC_CACHE:
        _NC_CACHE[reps] = build_nc(reps)
    return _NC_CACHE[reps]


def kernel(pred, target, mask):
    nc = _get_nc(1)
    in_maps = make_in_maps(pred, target, mask)
    res = run_bass_kernel_spmd(nc, in_maps, list(range(NCORES)))
    return combine(res.results)
